# revision 9
# baseline (speedup 1.0000x reference)
"""GrowingCrystalAttention Trainium2 kernel (split-precision fp8 rewrite).

Expert-parallel over 8 NeuronCores: each core handles 16 of the 128
"neurons" (experts).

Numerical split: softmax rows sum to 1, so with W_bar = mean_n(W_n),
  out = X @ W_bar @ outW^T + sum_n (attn[:,n] - 1/N) * (X @ W_n) @ outW^T
The first (mean) term carries ~99.8% of the magnitude and is computed
in f32r; the delta term is tiny, so its big contraction runs in
fp8e4m3 with DoubleRow perf mode (2 fp8 MACs per PE cell per cycle).
W is pre-scaled by 64 into fp8 range; delta absorbs the 1/64.

Per core:
  - attention: xp = X @ posT via fp8 matmuls, softmax on ACT+DVE in
    fp32, delta' = softmax/64 - 1/(N*64)
  - main loop: P_n = X @ (64 W_n) as fp8 DoubleRow matmuls,
    acc += delta'[:, n] * P_n via DVE(fused) / ACT+Pool drains
  - 4 bt-blocks, each ReduceScatter'd in bf16 (overlapped)
  - final per block: y = accT @ outW^T (bf16) + X_shard @ M2 (f32r) + b
    where M2 = W_bar @ outW^T is host-precomputed

SPMD trick: every core runs the identical program; per-core inputs are
permuted so that attention columns 0..15 are always the core's own
experts.
"""
import os
import sys

sys.path.insert(0, "/opt/trn_rl_repo")

import numpy as np
import ml_dtypes

import concourse.bass as bass
import concourse.mybir as mybir
import concourse.tile as tile
from concourse import bacc
from concourse.bass import ts, ds
from concourse.bass_utils import run_bass_kernel_spmd
from concourse.masks import make_identity

AF = mybir.ActivationFunctionType
ALU = mybir.AluOpType
F32 = mybir.dt.float32
F32R = mybir.dt.float32r
BF16 = mybir.dt.bfloat16
F8 = mybir.dt.float8e4
DR = mybir.MatmulPerfMode.DoubleRow
NPF8 = ml_dtypes.float8_e4m3
NPBF = ml_dtypes.bfloat16

NCORES = 8
B, T, D = 4, 512, 512
N = 128
BT = B * T            # 2048
NLOC = N // NCORES    # 16
NTILES = BT // 128    # 16
KCH = D // 128        # 4
NBLK = 4
TPB = NTILES // NBLK  # 4 tiles per block
ROWS = TPB * 128 // NCORES  # 64 output rows per core per block
WS = 64.0             # fp8 weight pre-scale
USE_DR = os.environ.get("K_NO_DR", "0") != "1"  # fp8 DoubleRow perf mode

_PROGRAM = None


def _build_program():
    nc = bacc.Bacc("TRN2", target_bir_lowering=False, debug=False,
                   num_devices=NCORES)

    xt8 = nc.dram_tensor("xt8", [128, KCH * BT], F8, kind="ExternalInput").ap()
    pos8 = nc.dram_tensor("pos8", [128, KCH * N], F8, kind="ExternalInput").ap()
    aug = nc.dram_tensor("aug", [1, N], BF16, kind="ExternalInput").ap()
    scb = nc.dram_tensor("scb", [128, N], F32, kind="ExternalInput").ap()
    x2 = nc.dram_tensor("x2", [BT, 1], F32, kind="ExternalInput").ap()
    w8 = nc.dram_tensor("w8", [128, NLOC * KCH * D], F8, kind="ExternalInput").ap()
    m2 = nc.dram_tensor("m2", [D, D], F32R, kind="ExternalInput").ap()
    owtb = nc.dram_tensor("owtb", [D, D], BF16, kind="ExternalInput").ap()
    obb = nc.dram_tensor("obb", [128, D], F32, kind="ExternalInput").ap()
    xs = nc.dram_tensor("xs", [NBLK * KCH * 128, ROWS], F32R,
                        kind="ExternalInput").ap()
    y = nc.dram_tensor("y", [NBLK * ROWS, D], F32, kind="ExternalOutput").ap()

    with tile.TileContext(nc) as tc:
        with tc.tile_pool(name="const", bufs=1) as constp, \
             tc.tile_pool(name="tmp", bufs=3) as tmpp, \
             tc.tile_pool(name="sc", bufs=4) as scp, \
             tc.tile_pool(name="stat", bufs=4) as statp, \
             tc.tile_pool(name="pmain", bufs=6, space="PSUM") as pmain, \
             tc.tile_pool(name="psmall", bufs=2, space="PSUM") as psmall, \
             tc.tile_pool(name="dram", bufs=1, space="DRAM") as dramp:

            # ---- persistent SBUF tiles + input DMAs ----
            # smalls first on the sync queue so attention starts ASAP
            pos8t = constp.tile([128, KCH, N], F8, tag="pos8", name="pos8")
            nc.sync.dma_start(pos8t[:], pos8[:])
            augt = constp.tile([1, N], BF16, tag="aug", name="aug")
            nc.sync.dma_start(augt[:], aug[:])
            ones = constp.tile([1, N], BF16, tag="ones", name="ones")
            nc.gpsimd.memset(ones[:], 1.0)
            scbt = constp.tile([128, N], F32, tag="scb", name="scb")
            nc.sync.dma_start(scbt[:], scb[:])
            x2t = [constp.tile([128, 1], F32, tag=f"x2_{i}", name=f"x2_{i}")
                   for i in range(NTILES)]
            for i in range(NTILES):
                nc.sync.dma_start(x2t[i][:], x2[ts(i, 128), :])
            xt8t = constp.tile([128, KCH, BT], F8, tag="xt8", name="xt8")
            for j in range(KCH):
                nc.sync.dma_start(xt8t[:, j, :], xt8[:, ts(j, BT)])
            # weights: 4 groups of 4 experts, resident for the whole kernel
            w8t = constp.tile([128, NLOC, KCH, D], F8, tag="w8", name="w8")
            for g in range(4):
                nc.sync.dma_start(w8t[:, ds(g * 4, 4), :, :],
                                  w8[:, ds(g * 4 * KCH * D, 4 * KCH * D)])
            # final-projection operands on the scalar (ACT) HWDGE queue
            m2t = [constp.tile([128, D], F32R, tag=f"m2_{k}", name=f"m2_{k}")
                   for k in range(KCH)]
            for k in range(KCH):
                nc.scalar.dma_start(m2t[k][:], m2[ts(k, 128), :])
            owtt = [constp.tile([128, D], BF16, tag=f"owt{e}", name=f"owt{e}")
                    for e in range(KCH)]
            for e in range(KCH):
                nc.scalar.dma_start(owtt[e][:], owtb[ts(e, 128), :])
            obbt = constp.tile([128, D], F32, tag="obb", name="obb")
            nc.scalar.dma_start(obbt[:], obb[:])
            xst = [[constp.tile([128, ROWS], F32R, tag=f"xs{b}_{k}",
                                name=f"xs{b}_{k}") for k in range(KCH)]
                   for b in range(NBLK)]
            for b in range(NBLK):
                for k in range(KCH):
                    nc.scalar.dma_start(xst[b][k][:],
                                        xs[ts(b * KCH + k, 128), :])

            acc = [constp.tile([128, D], F32, tag=f"acc{i}", name=f"acc{i}")
                   for i in range(NTILES)]
            attn = [constp.tile([128, N], F32, tag=f"attn{i}", name=f"attn{i}")
                    for i in range(NTILES)]

            # ---- stage A: attention -> delta' (all 16 bt tiles) ----
            for i in range(NTILES):
                xps = psmall.tile([128, N], F32, tag="xps", name="xps")
                for j in range(KCH):
                    nc.tensor.matmul(xps[:], xt8t[:, j, ts(i, 128)],
                                     pos8t[:, j, :],
                                     start=(j == 0), stop=False)
                nc.tensor.matmul(xps[:], ones[:], augt[:],
                                 start=False, stop=True)
                # dist = sqrt(x2 - 2*xp); den = dist + 0.1
                dist = tmpp.tile([128, N], F32, tag="dist", name="dist")
                nc.scalar.activation(dist[:], xps[:], AF.Sqrt,
                                     bias=x2t[i][:], scale=-2.0)
                nc.vector.tensor_scalar_add(dist[:], dist[:], 0.1)
                rec = tmpp.tile([128, N], F32, tag="rec", name="rec")
                nc.vector.reciprocal(rec[:], dist[:])
                # inter = rec * scales ; mx = rowmax(inter)   (one DVE op)
                inter = tmpp.tile([128, N], F32, tag="inter", name="inter")
                mx = statp.tile([128, 1], F32, tag="mx", name="mx")
                nc.vector.tensor_tensor_reduce(
                    inter[:], rec[:], scbt[:], 1.0, 0.0,
                    op0=ALU.mult, op1=ALU.max, accum_out=mx[:])
                negmx = statp.tile([128, 1], F32, tag="negmx", name="negmx")
                nc.vector.tensor_scalar_mul(negmx[:], mx[:], -1.0)
                # ex = exp(inter - mx) ; sm = rowsum(ex)   (one ACT op)
                ex = tmpp.tile([128, N], F32, tag="ex", name="ex")
                sm = statp.tile([128, 1], F32, tag="sm", name="sm")
                nc.scalar.activation(ex[:], inter[:], AF.Exp,
                                     bias=negmx[:], scale=1.0,
                                     accum_out=sm[:])
                rs1 = statp.tile([128, 1], F32, tag="rs1", name="rs1")
                nc.vector.reciprocal(rs1[:], sm[:])
                rs2 = statp.tile([128, 1], F32, tag="rs2", name="rs2")
                nc.vector.tensor_scalar_mul(rs2[:], rs1[:], 1.0 / WS)
                # delta' = ex * rs2 - 1/(N*WS)   (one DVE op)
                nc.vector.tensor_scalar(attn[i][:], ex[:], rs2[:],
                                        -1.0 / (N * WS),
                                        op0=ALU.mult, op1=ALU.add)

            # ---- stage B: fp8 DoubleRow expert matmuls + drains ----
            partial = [dramp.tile([TPB * 128, D], BF16, tag=f"part{b}",
                                  name=f"part{b}") for b in range(NBLK)]
            rs_out = [dramp.tile([ROWS, D], BF16, tag=f"rso{b}",
                                 name=f"rso{b}") for b in range(NBLK)]

            for b in range(NBLK):
                i0 = b * TPB
                for n in range(NLOC):
                    for t in range(TPB):
                        i = i0 + t
                        pp = pmain.tile([128, D], F32, tag="pm", name="pm")
                        if USE_DR:
                            for j in range(2):
                                nc.tensor.matmul(
                                    pp[:],
                                    xt8t[:, ds(2 * j, 2), ts(i, 128)],
                                    w8t[:, n, ds(2 * j, 2), :],
                                    start=(j == 0), stop=(j == 1),
                                    perf_mode=DR)
                        else:
                            for j in range(KCH):
                                nc.tensor.matmul(
                                    pp[:],
                                    xt8t[:, j, ts(i, 128)],
                                    w8t[:, n, j, :],
                                    start=(j == 0), stop=(j == KCH - 1))
                        col = attn[i][:, n:n + 1]
                        # drain: acc += delta' * P, spread across engines
                        if n == 0:
                            nc.scalar.activation(acc[i][:], pp[:], AF.Copy,
                                                 scale=col)
                        elif (n * TPB + t) % 5 < 3:
                            nc.vector.scalar_tensor_tensor(
                                acc[i][:], pp[:], col, acc[i][:],
                                op0=ALU.mult, op1=ALU.add)
                        else:
                            sc = scp.tile([128, D], F32, tag="sc", name="sc")
                            nc.scalar.activation(sc[:], pp[:], AF.Copy,
                                                 scale=col)
                            nc.gpsimd.tensor_add(acc[i][:], acc[i][:], sc[:])
                # block done: cast-store partials (f32 -> bf16) + RS
                for t in range(TPB):
                    nc.gpsimd.dma_start(partial[b][ts(t, 128), :],
                                        acc[i0 + t][:])
                nc.gpsimd.collective_compute(
                    "ReduceScatter",
                    ALU.add,
                    replica_groups=[list(range(NCORES))],
                    ins=[partial[b][:]],
                    outs=[rs_out[b][:]],
                )

            # ---- stage C: final projection per block ----
            identb = constp.tile([128, 128], BF16, tag="identb", name="identb")
            make_identity(nc, identb[:])
            for b in range(NBLK):
                yacc = constp.tile([128, D], BF16, tag=f"yacc{b}",
                                   name=f"yacc{b}")
                nc.sync.dma_start(yacc[:ROWS, :], rs_out[b][:])
                yt = [constp.tile([128, ROWS], BF16, tag=f"yt{b}_{e}",
                                  name=f"yt{b}_{e}") for e in range(KCH)]
                for e in range(KCH):
                    pt = pmain.tile([128, 128], BF16, tag="pm", name="pm")
                    nc.tensor.transpose(pt[:, :ROWS], yacc[:ROWS, ts(e, 128)],
                                        identb[:ROWS, :ROWS])
                    nc.vector.tensor_copy(yt[e][:, :ROWS], pt[:, :ROWS])
                po = pmain.tile([128, D], F32, tag="pm", name="pm")
                for e in range(KCH):
                    nc.tensor.matmul(po[:ROWS, :], yt[e][:, :ROWS], owtt[e][:],
                                     start=(e == 0), stop=False)
                for k in range(KCH):
                    nc.tensor.matmul(po[:ROWS, :], xst[b][k][:], m2t[k][:],
                                     start=False, stop=(k == KCH - 1))
                yo = constp.tile([128, D], F32, tag=f"yo{b}", name=f"yo{b}")
                nc.vector.tensor_add(yo[:ROWS, :], po[:ROWS, :],
                                     obbt[:ROWS, :])
                nc.sync.dma_start(y[ts(b, ROWS), :], yo[:ROWS, :])

    nc.compile()
    return nc


def kernel(x, positions, scales, value_weight, out_W, out_b):
    global _PROGRAM
    if _PROGRAM is None:
        _PROGRAM = _build_program()
    nc = _PROGRAM

    X = np.ascontiguousarray(np.asarray(x, np.float32).reshape(BT, D))
    XT = np.ascontiguousarray(X.T)                       # (D, BT)
    # [p, j, bt] = X[bt, j*128+p]
    xt8_h = np.ascontiguousarray(
        XT.reshape(KCH, 128, BT).transpose(1, 0, 2)
    ).astype(NPF8).reshape(128, KCH * BT)
    x2 = (X.astype(np.float64) ** 2).sum(1).astype(np.float32).reshape(BT, 1)
    pos = np.asarray(positions, np.float32)
    pn2 = (pos.astype(np.float64) ** 2).sum(1)           # (N,)
    sc = np.asarray(scales, np.float32)
    vw_full = np.asarray(value_weight, np.float32)
    owt = np.asarray(out_W, np.float32).T                # [e, d2]
    owtb_h = np.ascontiguousarray(owt).astype(NPBF)
    obb_h = np.tile(np.asarray(out_b, np.float32), (128, 1))
    wbar = vw_full.mean(axis=0, dtype=np.float64)        # (D, D)
    m2_h = np.ascontiguousarray(
        (wbar @ owt.astype(np.float64)).astype(np.float32))

    in_maps = []
    for c in range(NCORES):
        mine = np.arange(c * NLOC, (c + 1) * NLOC)
        rest = np.delete(np.arange(N), mine)
        perm = np.concatenate([mine, rest])
        posT = pos[perm].T                               # (D, N)
        pos8_h = np.ascontiguousarray(
            posT.reshape(KCH, 128, N).transpose(1, 0, 2)
        ).astype(NPF8).reshape(128, KCH * N)
        # [p, n, j, e] = 64 * W_perm[n][j*128+p, e]
        wloc = vw_full[mine] * WS                        # (16, 512, 512)
        w8_h = np.ascontiguousarray(
            wloc.reshape(NLOC, KCH, 128, D).transpose(2, 0, 1, 3)
        ).astype(NPF8).reshape(128, NLOC * KCH * D)
        # xs[b, k, p, r] = X[b*512 + c*64 + r, k*128 + p]
        xs_h = np.empty((NBLK, KCH, 128, ROWS), np.float32)
        for b in range(NBLK):
            sh = X[b * TPB * 128 + c * ROWS:
                   b * TPB * 128 + (c + 1) * ROWS, :]    # (64, 512)
            xs_h[b] = sh.T.reshape(KCH, 128, ROWS)
        in_maps.append({
            "xt8": xt8_h,
            "pos8": pos8_h,
            "aug": (-0.5 * pn2[perm]).astype(np.float32).astype(
                NPBF).reshape(1, N),
            "scb": np.tile(sc[perm], (128, 1)).astype(np.float32),
            "x2": x2,
            "w8": w8_h,
            "m2": m2_h,
            "owtb": owtb_h,
            "obb": obb_h,
            "xs": xs_h.reshape(NBLK * KCH * 128, ROWS),
        })

    trace = os.environ.get("BASS_KERNEL_TRACE", "0") == "1"
    res = run_bass_kernel_spmd(nc, in_maps, core_ids=list(range(NCORES)),
                               trace=trace)
    if trace:
        kernel.last_exec_time_ns = res.exec_time_ns
        kernel.last_trace = (res.instructions_and_trace or (None, None))[1]

    yfull = np.empty((BT, D), np.float32)
    for r in range(NCORES):
        yr = res.results[r]["y"]
        for b in range(NBLK):
            g0 = b * TPB * 128 + r * ROWS
            yfull[g0:g0 + ROWS] = yr[b * ROWS:(b + 1) * ROWS]
    return yfull.reshape(B, T, D)


# revision 10
# speedup vs baseline: 1.9150x; 1.9150x over previous
"""GrowingCrystalAttention Trainium2 kernel (split-precision fp8 rewrite).

Expert-parallel over 8 NeuronCores: each core handles 16 of the 128
"neurons" (experts).

Numerical split: softmax rows sum to 1, so with W_bar = mean_n(W_n),
  out = X @ W_bar @ outW^T + sum_n (attn[:,n] - 1/N) * (X @ W_n) @ outW^T
The first (mean) term carries ~99.8% of the magnitude and is computed
in f32r; the delta term is tiny, so its big contraction runs in
fp8e4m3 with DoubleRow perf mode (2 fp8 MACs per PE cell per cycle).
W is pre-scaled by 64 into fp8 range; delta absorbs the 1/64.

Per core:
  - attention: xp = X @ posT via fp8 matmuls, softmax on ACT+DVE in
    fp32, delta' = softmax/64 - 1/(N*64)
  - main loop: P_n = X @ (64 W_n) as fp8 DoubleRow matmuls,
    acc += delta'[:, n] * P_n via DVE(fused) / ACT+Pool drains
  - 4 bt-blocks, each ReduceScatter'd in bf16 (overlapped)
  - final per block: y = accT @ outW^T (bf16) + X_shard @ M2 (f32r) + b
    where M2 = W_bar @ outW^T is host-precomputed

SPMD trick: every core runs the identical program; per-core inputs are
permuted so that attention columns 0..15 are always the core's own
experts.
"""
import os
import sys

sys.path.insert(0, "/opt/trn_rl_repo")

import numpy as np
import ml_dtypes

import concourse.bass as bass
import concourse.mybir as mybir
import concourse.tile as tile
from concourse import bacc
from concourse.bass import ts, ds
from concourse.bass_utils import run_bass_kernel_spmd
from concourse.masks import make_identity

AF = mybir.ActivationFunctionType
ALU = mybir.AluOpType
F32 = mybir.dt.float32
F32R = mybir.dt.float32r
BF16 = mybir.dt.bfloat16
F8 = mybir.dt.float8e4
DR = mybir.MatmulPerfMode.DoubleRow
NPF8 = ml_dtypes.float8_e4m3
NPBF = ml_dtypes.bfloat16

NCORES = 8
B, T, D = 4, 512, 512
N = 128
BT = B * T            # 2048
NLOC = N // NCORES    # 16
NTILES = BT // 128    # 16
KCH = D // 128        # 4
NBLK = 4
TPB = NTILES // NBLK  # 4 tiles per block
ROWS = TPB * 128 // NCORES  # 64 output rows per core per block
WS = 64.0             # fp8 weight pre-scale
USE_DR = os.environ.get("K_NO_DR", "0") != "1"  # fp8 DoubleRow perf mode

_PROGRAM = None


def _build_program():
    nc = bacc.Bacc("TRN2", target_bir_lowering=False, debug=False,
                   num_devices=NCORES)

    xt8 = nc.dram_tensor("xt8", [128, KCH * BT], F8, kind="ExternalInput").ap()
    pos8 = nc.dram_tensor("pos8", [128, KCH * N], F8, kind="ExternalInput").ap()
    aug = nc.dram_tensor("aug", [1, N], BF16, kind="ExternalInput").ap()
    scb = nc.dram_tensor("scb", [128, N], F32, kind="ExternalInput").ap()
    x2 = nc.dram_tensor("x2", [BT, 1], F32, kind="ExternalInput").ap()
    w8 = nc.dram_tensor("w8", [128, NLOC * KCH * D], F8, kind="ExternalInput").ap()
    m2 = nc.dram_tensor("m2", [D, D], F32R, kind="ExternalInput").ap()
    owtb = nc.dram_tensor("owtb", [D, D], BF16, kind="ExternalInput").ap()
    obb = nc.dram_tensor("obb", [128, D], F32, kind="ExternalInput").ap()
    xs = nc.dram_tensor("xs", [NBLK * KCH * 128, ROWS], F32R,
                        kind="ExternalInput").ap()
    y = nc.dram_tensor("y", [NBLK * ROWS, D], F32, kind="ExternalOutput").ap()

    with tile.TileContext(nc) as tc:
        with tc.tile_pool(name="const", bufs=1) as constp, \
             tc.tile_pool(name="tmp", bufs=3) as tmpp, \
             tc.tile_pool(name="sc", bufs=4) as scp, \
             tc.tile_pool(name="stat", bufs=4) as statp, \
             tc.tile_pool(name="pmain", bufs=6, space="PSUM") as pmain, \
             tc.tile_pool(name="psmall", bufs=2, space="PSUM") as psmall, \
             tc.tile_pool(name="dram", bufs=1, space="DRAM") as dramp:

            # ---- persistent SBUF tiles + input DMAs ----
            # smalls first on the sync queue so attention starts ASAP
            pos8t = constp.tile([128, KCH, N], F8, tag="pos8", name="pos8")
            nc.sync.dma_start(pos8t[:], pos8[:])
            augt = constp.tile([1, N], BF16, tag="aug", name="aug")
            nc.sync.dma_start(augt[:], aug[:])
            ones = constp.tile([1, N], BF16, tag="ones", name="ones")
            nc.gpsimd.memset(ones[:], 1.0)
            scbt = constp.tile([128, N], F32, tag="scb", name="scb")
            nc.sync.dma_start(scbt[:], scb[:])
            x2t = [constp.tile([128, 1], F32, tag=f"x2_{i}", name=f"x2_{i}")
                   for i in range(NTILES)]
            for i in range(NTILES):
                nc.sync.dma_start(x2t[i][:], x2[ts(i, 128), :])
            xt8t = constp.tile([128, KCH, BT], F8, tag="xt8", name="xt8")
            for j in range(KCH):
                nc.sync.dma_start(xt8t[:, j, :], xt8[:, ts(j, BT)])
            # weights: 4 groups of 4 experts, resident for the whole kernel
            w8t = constp.tile([128, NLOC, KCH, D], F8, tag="w8", name="w8")
            for g in range(4):
                nc.sync.dma_start(w8t[:, ds(g * 4, 4), :, :],
                                  w8[:, ds(g * 4 * KCH * D, 4 * KCH * D)])
            # final-projection operands on the scalar (ACT) HWDGE queue
            m2t = [constp.tile([128, D], F32R, tag=f"m2_{k}", name=f"m2_{k}")
                   for k in range(KCH)]
            for k in range(KCH):
                nc.scalar.dma_start(m2t[k][:], m2[ts(k, 128), :])
            owtt = [constp.tile([128, D], BF16, tag=f"owt{e}", name=f"owt{e}")
                    for e in range(KCH)]
            for e in range(KCH):
                nc.scalar.dma_start(owtt[e][:], owtb[ts(e, 128), :])
            obbt = constp.tile([128, D], F32, tag="obb", name="obb")
            nc.scalar.dma_start(obbt[:], obb[:])
            xst = [[constp.tile([128, ROWS], F32R, tag=f"xs{b}_{k}",
                                name=f"xs{b}_{k}") for k in range(KCH)]
                   for b in range(NBLK)]
            for b in range(NBLK):
                for k in range(KCH):
                    nc.scalar.dma_start(xst[b][k][:],
                                        xs[ts(b * KCH + k, 128), :])

            acc = [constp.tile([128, D], F32, tag=f"acc{i}", name=f"acc{i}")
                   for i in range(NTILES)]
            attn = [constp.tile([128, N], F32, tag=f"attn{i}", name=f"attn{i}")
                    for i in range(NTILES)]

            # ---- stage A: attention -> delta' (all 16 bt tiles) ----
            for i in range(NTILES):
                xps = psmall.tile([128, N], F32, tag="xps", name="xps")
                for j in range(KCH):
                    nc.tensor.matmul(xps[:], xt8t[:, j, ts(i, 128)],
                                     pos8t[:, j, :],
                                     start=(j == 0), stop=False)
                nc.tensor.matmul(xps[:], ones[:], augt[:],
                                 start=False, stop=True)
                # dist = sqrt(x2 - 2*xp); den = dist + 0.1
                dist = tmpp.tile([128, N], F32, tag="dist", name="dist")
                nc.scalar.activation(dist[:], xps[:], AF.Sqrt,
                                     bias=x2t[i][:], scale=-2.0)
                nc.vector.tensor_scalar_add(dist[:], dist[:], 0.1)
                rec = tmpp.tile([128, N], F32, tag="rec", name="rec")
                nc.vector.reciprocal(rec[:], dist[:])
                inter = tmpp.tile([128, N], F32, tag="inter", name="inter")
                nc.vector.tensor_mul(inter[:], rec[:], scbt[:])
                mx = statp.tile([128, 1], F32, tag="mx", name="mx")
                nc.vector.tensor_reduce(mx[:], inter[:],
                                        axis=mybir.AxisListType.X,
                                        op=ALU.max)
                negmx = statp.tile([128, 1], F32, tag="negmx", name="negmx")
                nc.vector.tensor_scalar_mul(negmx[:], mx[:], -1.0)
                ex = tmpp.tile([128, N], F32, tag="ex", name="ex")
                nc.scalar.activation(ex[:], inter[:], AF.Exp,
                                     bias=negmx[:], scale=1.0)
                sm = statp.tile([128, 1], F32, tag="sm", name="sm")
                nc.vector.tensor_reduce(sm[:], ex[:],
                                        axis=mybir.AxisListType.X,
                                        op=ALU.add)
                rs1 = statp.tile([128, 1], F32, tag="rs1", name="rs1")
                nc.vector.reciprocal(rs1[:], sm[:])
                rs2 = statp.tile([128, 1], F32, tag="rs2", name="rs2")
                nc.vector.tensor_scalar_mul(rs2[:], rs1[:], 1.0 / WS)
                # delta' = ex * rs2 - 1/(N*WS)   (one DVE op)
                nc.vector.tensor_scalar(attn[i][:], ex[:], rs2[:],
                                        -1.0 / (N * WS),
                                        op0=ALU.mult, op1=ALU.add)

            # ---- stage B: fp8 DoubleRow expert matmuls + drains ----
            partial = [dramp.tile([TPB * 128, D], BF16, tag=f"part{b}",
                                  name=f"part{b}") for b in range(NBLK)]
            rs_out = [dramp.tile([ROWS, D], BF16, tag=f"rso{b}",
                                 name=f"rso{b}") for b in range(NBLK)]

            for b in range(NBLK):
                i0 = b * TPB
                for n in range(NLOC):
                    for t in range(TPB):
                        i = i0 + t
                        pp = pmain.tile([128, D], F32, tag="pm", name="pm")
                        if USE_DR:
                            for j in range(2):
                                nc.tensor.matmul(
                                    pp[:],
                                    xt8t[:, ds(2 * j, 2), ts(i, 128)],
                                    w8t[:, n, ds(2 * j, 2), :],
                                    start=(j == 0), stop=(j == 1),
                                    perf_mode=DR)
                        else:
                            for j in range(KCH):
                                nc.tensor.matmul(
                                    pp[:],
                                    xt8t[:, j, ts(i, 128)],
                                    w8t[:, n, j, :],
                                    start=(j == 0), stop=(j == KCH - 1))
                        col = attn[i][:, n:n + 1]
                        # drain: acc += delta' * P, spread across engines
                        if n == 0:
                            nc.scalar.activation(acc[i][:], pp[:], AF.Copy,
                                                 scale=col)
                        elif (n * TPB + t) % 5 < 3:
                            nc.vector.scalar_tensor_tensor(
                                acc[i][:], pp[:], col, acc[i][:],
                                op0=ALU.mult, op1=ALU.add)
                        else:
                            sc = scp.tile([128, D], F32, tag="sc", name="sc")
                            nc.scalar.activation(sc[:], pp[:], AF.Copy,
                                                 scale=col)
                            nc.gpsimd.tensor_add(acc[i][:], acc[i][:], sc[:])
                # block done: cast-store partials (f32 -> bf16) + RS
                for t in range(TPB):
                    nc.gpsimd.dma_start(partial[b][ts(t, 128), :],
                                        acc[i0 + t][:])
                nc.gpsimd.collective_compute(
                    "ReduceScatter",
                    ALU.add,
                    replica_groups=[list(range(NCORES))],
                    ins=[partial[b][:]],
                    outs=[rs_out[b][:]],
                )

            # ---- stage C: final projection per block ----
            identb = constp.tile([128, 128], BF16, tag="identb", name="identb")
            make_identity(nc, identb[:])
            for b in range(NBLK):
                yacc = constp.tile([128, D], BF16, tag=f"yacc{b}",
                                   name=f"yacc{b}")
                nc.sync.dma_start(yacc[:ROWS, :], rs_out[b][:])
                yt = [constp.tile([128, ROWS], BF16, tag=f"yt{b}_{e}",
                                  name=f"yt{b}_{e}") for e in range(KCH)]
                for e in range(KCH):
                    pt = pmain.tile([128, 128], BF16, tag="pm", name="pm")
                    nc.tensor.transpose(pt[:, :ROWS], yacc[:ROWS, ts(e, 128)],
                                        identb[:ROWS, :ROWS])
                    nc.vector.tensor_copy(yt[e][:, :ROWS], pt[:, :ROWS])
                po = pmain.tile([128, D], F32, tag="pm", name="pm")
                for e in range(KCH):
                    nc.tensor.matmul(po[:ROWS, :], yt[e][:, :ROWS], owtt[e][:],
                                     start=(e == 0), stop=False)
                for k in range(KCH):
                    nc.tensor.matmul(po[:ROWS, :], xst[b][k][:], m2t[k][:],
                                     start=False, stop=(k == KCH - 1))
                yo = constp.tile([128, D], F32, tag=f"yo{b}", name=f"yo{b}")
                nc.vector.tensor_add(yo[:ROWS, :], po[:ROWS, :],
                                     obbt[:ROWS, :])
                nc.sync.dma_start(y[ts(b, ROWS), :], yo[:ROWS, :])

    nc.compile()
    return nc


def kernel(x, positions, scales, value_weight, out_W, out_b):
    global _PROGRAM
    if _PROGRAM is None:
        _PROGRAM = _build_program()
    nc = _PROGRAM

    X = np.ascontiguousarray(np.asarray(x, np.float32).reshape(BT, D))
    XT = np.ascontiguousarray(X.T)                       # (D, BT)
    # [p, j, bt] = X[bt, j*128+p]
    xt8_h = np.ascontiguousarray(
        XT.reshape(KCH, 128, BT).transpose(1, 0, 2)
    ).astype(NPF8).reshape(128, KCH * BT)
    x2 = (X.astype(np.float64) ** 2).sum(1).astype(np.float32).reshape(BT, 1)
    pos = np.asarray(positions, np.float32)
    pn2 = (pos.astype(np.float64) ** 2).sum(1)           # (N,)
    sc = np.asarray(scales, np.float32)
    vw_full = np.asarray(value_weight, np.float32)
    owt = np.asarray(out_W, np.float32).T                # [e, d2]
    owtb_h = np.ascontiguousarray(owt).astype(NPBF)
    obb_h = np.tile(np.asarray(out_b, np.float32), (128, 1))
    wbar = vw_full.mean(axis=0, dtype=np.float64)        # (D, D)
    m2_h = np.ascontiguousarray(
        (wbar @ owt.astype(np.float64)).astype(np.float32))

    in_maps = []
    for c in range(NCORES):
        mine = np.arange(c * NLOC, (c + 1) * NLOC)
        rest = np.delete(np.arange(N), mine)
        perm = np.concatenate([mine, rest])
        posT = pos[perm].T                               # (D, N)
        pos8_h = np.ascontiguousarray(
            posT.reshape(KCH, 128, N).transpose(1, 0, 2)
        ).astype(NPF8).reshape(128, KCH * N)
        # [p, n, j, e] = 64 * W_perm[n][j*128+p, e]
        wloc = vw_full[mine] * WS                        # (16, 512, 512)
        w8_h = np.ascontiguousarray(
            wloc.reshape(NLOC, KCH, 128, D).transpose(2, 0, 1, 3)
        ).astype(NPF8).reshape(128, NLOC * KCH * D)
        # xs[b, k, p, r] = X[b*512 + c*64 + r, k*128 + p]
        xs_h = np.empty((NBLK, KCH, 128, ROWS), np.float32)
        for b in range(NBLK):
            sh = X[b * TPB * 128 + c * ROWS:
                   b * TPB * 128 + (c + 1) * ROWS, :]    # (64, 512)
            xs_h[b] = sh.T.reshape(KCH, 128, ROWS)
        in_maps.append({
            "xt8": xt8_h,
            "pos8": pos8_h,
            "aug": (-0.5 * pn2[perm]).astype(np.float32).astype(
                NPBF).reshape(1, N),
            "scb": np.tile(sc[perm], (128, 1)).astype(np.float32),
            "x2": x2,
            "w8": w8_h,
            "m2": m2_h,
            "owtb": owtb_h,
            "obb": obb_h,
            "xs": xs_h.reshape(NBLK * KCH * 128, ROWS),
        })

    trace = os.environ.get("BASS_KERNEL_TRACE", "0") == "1"
    res = run_bass_kernel_spmd(nc, in_maps, core_ids=list(range(NCORES)),
                               trace=trace)
    if trace:
        kernel.last_exec_time_ns = res.exec_time_ns
        kernel.last_trace = (res.instructions_and_trace or (None, None))[1]

    yfull = np.empty((BT, D), np.float32)
    for r in range(NCORES):
        yr = res.results[r]["y"]
        for b in range(NBLK):
            g0 = b * TPB * 128 + r * ROWS
            yfull[g0:g0 + ROWS] = yr[b * ROWS:(b + 1) * ROWS]
    return yfull.reshape(B, T, D)


# revision 12
# speedup vs baseline: 1.9348x; 1.0103x over previous
"""GrowingCrystalAttention Trainium2 kernel (split-precision fp8 rewrite).

Expert-parallel over 8 NeuronCores: each core handles 16 of the 128
"neurons" (experts).

Numerical split: softmax rows sum to 1, so with W_bar = mean_n(W_n),
  out = X @ W_bar @ outW^T + sum_n (attn[:,n] - 1/N) * (X @ W_n) @ outW^T
The first (mean) term carries ~99.8% of the magnitude and is computed
in f32r; the delta term is tiny, so its big contraction runs in
fp8e4m3 with DoubleRow perf mode (2 fp8 MACs per PE cell per cycle).
W is pre-scaled by 64 into fp8 range; delta absorbs the 1/64.

Per core:
  - attention: xp = X @ posT via fp8 matmuls, softmax on ACT+DVE in
    fp32, delta' = softmax/64 - 1/(N*64)
  - main loop: P_n = X @ (64 W_n) as fp8 DoubleRow matmuls,
    acc += delta'[:, n] * P_n via DVE(fused) / ACT+Pool drains
  - 4 bt-blocks, each ReduceScatter'd in bf16 (overlapped)
  - final per block: y = accT @ outW^T (bf16) + X_shard @ M2 (f32r) + b
    where M2 = W_bar @ outW^T is host-precomputed

SPMD trick: every core runs the identical program; per-core inputs are
permuted so that attention columns 0..15 are always the core's own
experts.
"""
import os
import sys

sys.path.insert(0, "/opt/trn_rl_repo")

import numpy as np
import ml_dtypes

import concourse.bass as bass
import concourse.mybir as mybir
import concourse.tile as tile
from concourse import bacc
from concourse.bass import ts, ds
from concourse.bass_utils import run_bass_kernel_spmd
from concourse.masks import make_identity

AF = mybir.ActivationFunctionType
ALU = mybir.AluOpType
F32 = mybir.dt.float32
F32R = mybir.dt.float32r
BF16 = mybir.dt.bfloat16
F8 = mybir.dt.float8e4
DR = mybir.MatmulPerfMode.DoubleRow
NPF8 = ml_dtypes.float8_e4m3
NPBF = ml_dtypes.bfloat16

NCORES = 8
B, T, D = 4, 512, 512
N = 128
BT = B * T            # 2048
NLOC = N // NCORES    # 16
NTILES = BT // 128    # 16
KCH = D // 128        # 4
NBLK = 4
TPB = NTILES // NBLK  # 4 tiles per block
ROWS = TPB * 128 // NCORES  # 64 output rows per core per block
WS = 64.0             # fp8 weight pre-scale
USE_DR = os.environ.get("K_NO_DR", "0") != "1"  # fp8 DoubleRow perf mode

_PROGRAM = None


def _build_program():
    nc = bacc.Bacc("TRN2", target_bir_lowering=False, debug=False,
                   num_devices=NCORES)

    xt8 = nc.dram_tensor("xt8", [128, KCH * BT], F8, kind="ExternalInput").ap()
    pos8 = nc.dram_tensor("pos8", [128, KCH * N], F8, kind="ExternalInput").ap()
    aug = nc.dram_tensor("aug", [1, N], BF16, kind="ExternalInput").ap()
    scb = nc.dram_tensor("scb", [128, N], F32, kind="ExternalInput").ap()
    x2 = nc.dram_tensor("x2", [BT, 1], F32, kind="ExternalInput").ap()
    w8 = nc.dram_tensor("w8", [128, NLOC * KCH * D], F8, kind="ExternalInput").ap()
    m2 = nc.dram_tensor("m2", [D, D], F32R, kind="ExternalInput").ap()
    owtb = nc.dram_tensor("owtb", [D, D], BF16, kind="ExternalInput").ap()
    obb = nc.dram_tensor("obb", [128, D], F32, kind="ExternalInput").ap()
    xs = nc.dram_tensor("xs", [NBLK * KCH * 128, ROWS], F32R,
                        kind="ExternalInput").ap()
    y = nc.dram_tensor("y", [NBLK * ROWS, D], F32, kind="ExternalOutput").ap()

    with tile.TileContext(nc) as tc:
        with tc.tile_pool(name="const", bufs=1) as constp, \
             tc.tile_pool(name="tmp", bufs=3) as tmpp, \
             tc.tile_pool(name="sc", bufs=4) as scp, \
             tc.tile_pool(name="stat", bufs=4) as statp, \
             tc.tile_pool(name="pmain", bufs=6, space="PSUM") as pmain, \
             tc.tile_pool(name="psmall", bufs=2, space="PSUM") as psmall, \
             tc.tile_pool(name="dram", bufs=1, space="DRAM") as dramp:

            # ---- persistent SBUF tiles + input DMAs ----
            # smalls first on the sync queue so attention starts ASAP
            pos8t = constp.tile([128, KCH, N], F8, tag="pos8", name="pos8")
            nc.sync.dma_start(pos8t[:], pos8[:])
            augt = constp.tile([1, N], BF16, tag="aug", name="aug")
            nc.sync.dma_start(augt[:], aug[:])
            ones = constp.tile([1, N], BF16, tag="ones", name="ones")
            nc.gpsimd.memset(ones[:], 1.0)
            scbt = constp.tile([128, N], F32, tag="scb", name="scb")
            nc.sync.dma_start(scbt[:], scb[:])
            x2t = [constp.tile([128, 1], F32, tag=f"x2_{i}", name=f"x2_{i}")
                   for i in range(NTILES)]
            for i in range(NTILES):
                nc.sync.dma_start(x2t[i][:], x2[ts(i, 128), :])
            xt8t = constp.tile([128, KCH, BT], F8, tag="xt8", name="xt8")
            for j in range(KCH):
                nc.sync.dma_start(xt8t[:, j, :], xt8[:, ts(j, BT)])
            # weights: 4 groups of 4 experts, resident for the whole kernel
            w8t = constp.tile([128, NLOC, KCH, D], F8, tag="w8", name="w8")
            for g in range(4):
                nc.sync.dma_start(w8t[:, ds(g * 4, 4), :, :],
                                  w8[:, ds(g * 4 * KCH * D, 4 * KCH * D)])
            # final-projection operands on the scalar (ACT) HWDGE queue
            m2t = [constp.tile([128, D], F32R, tag=f"m2_{k}", name=f"m2_{k}")
                   for k in range(KCH)]
            for k in range(KCH):
                nc.scalar.dma_start(m2t[k][:], m2[ts(k, 128), :])
            owtt = [constp.tile([128, D], BF16, tag=f"owt{e}", name=f"owt{e}")
                    for e in range(KCH)]
            for e in range(KCH):
                nc.scalar.dma_start(owtt[e][:], owtb[ts(e, 128), :])
            obbt = constp.tile([128, D], F32, tag="obb", name="obb")
            nc.scalar.dma_start(obbt[:], obb[:])
            xst = [[constp.tile([128, ROWS], F32R, tag=f"xs{b}_{k}",
                                name=f"xs{b}_{k}") for k in range(KCH)]
                   for b in range(NBLK)]
            for b in range(NBLK):
                for k in range(KCH):
                    nc.scalar.dma_start(xst[b][k][:],
                                        xs[ts(b * KCH + k, 128), :])

            acc = [constp.tile([128, D], BF16, tag=f"acc{i}", name=f"acc{i}")
                   for i in range(NTILES)]
            attn = [constp.tile([128, N], F32, tag=f"attn{i}", name=f"attn{i}")
                    for i in range(NTILES)]

            # ---- stage A: attention -> delta' ----
            # Two sweeps per 8-tile half (all Sqrt ops batched, then all
            # Exp ops batched) so the ACT engine doesn't thrash its
            # activation-function tables on every tile.
            inters = [constp.tile([128, N], F32, tag=f"int{i}", name=f"int{i}")
                      for i in range(NTILES)]
            exs = [constp.tile([128, N], F32, tag=f"ex{i}", name=f"ex{i}")
                   for i in range(NTILES)]
            negmxs = [statp.tile([128, 1], F32, tag=f"nmx{i}", name=f"nmx{i}")
                      for i in range(NTILES)]
            for h in range(2):
                tiles = range(h * 8, (h + 1) * 8)
                for i in tiles:
                    xps = psmall.tile([128, N], F32, tag="xps", name="xps")
                    for j in range(KCH):
                        nc.tensor.matmul(xps[:], xt8t[:, j, ts(i, 128)],
                                         pos8t[:, j, :],
                                         start=(j == 0), stop=False)
                    nc.tensor.matmul(xps[:], ones[:], augt[:],
                                     start=False, stop=True)
                    # dist = sqrt(x2 - 2*xp)
                    dist = tmpp.tile([128, N], F32, tag="dist", name="dist")
                    nc.scalar.activation(dist[:], xps[:], AF.Sqrt,
                                         bias=x2t[i][:], scale=-2.0)
                    nc.vector.tensor_scalar_add(dist[:], dist[:], 0.1)
                    rec = tmpp.tile([128, N], F32, tag="rec", name="rec")
                    nc.vector.reciprocal(rec[:], dist[:])
                    nc.vector.tensor_mul(inters[i][:], rec[:], scbt[:])
                    mx = statp.tile([128, 1], F32, tag="mx", name="mx")
                    nc.vector.tensor_reduce(mx[:], inters[i][:],
                                            axis=mybir.AxisListType.X,
                                            op=ALU.max)
                    nc.vector.tensor_scalar_mul(negmxs[i][:], mx[:], -1.0)
                for i in tiles:
                    nc.scalar.activation(exs[i][:], inters[i][:], AF.Exp,
                                         bias=negmxs[i][:], scale=1.0)
                    sm = statp.tile([128, 1], F32, tag="sm", name="sm")
                    nc.vector.tensor_reduce(sm[:], exs[i][:],
                                            axis=mybir.AxisListType.X,
                                            op=ALU.add)
                    rs1 = statp.tile([128, 1], F32, tag="rs1", name="rs1")
                    nc.vector.reciprocal(rs1[:], sm[:])
                    rs2 = statp.tile([128, 1], F32, tag="rs2", name="rs2")
                    nc.vector.tensor_scalar_mul(rs2[:], rs1[:], 1.0 / WS)
                    # delta' = ex * rs2 - 1/(N*WS)   (one DVE op)
                    nc.vector.tensor_scalar(attn[i][:], exs[i][:], rs2[:],
                                            -1.0 / (N * WS),
                                            op0=ALU.mult, op1=ALU.add)

            # ---- stage B: fp8 DoubleRow expert matmuls + drains ----
            partial = [dramp.tile([TPB * 128, D], BF16, tag=f"part{b}",
                                  name=f"part{b}") for b in range(NBLK)]
            rs_out = [dramp.tile([ROWS, D], BF16, tag=f"rso{b}",
                                 name=f"rso{b}") for b in range(NBLK)]

            for b in range(NBLK):
                i0 = b * TPB
                for n in range(NLOC):
                    for t in range(TPB):
                        i = i0 + t
                        pp = pmain.tile([128, D], F32, tag="pm", name="pm")
                        if USE_DR:
                            for j in range(2):
                                nc.tensor.matmul(
                                    pp[:],
                                    xt8t[:, ds(2 * j, 2), ts(i, 128)],
                                    w8t[:, n, ds(2 * j, 2), :],
                                    start=(j == 0), stop=(j == 1),
                                    perf_mode=DR)
                        else:
                            for j in range(KCH):
                                nc.tensor.matmul(
                                    pp[:],
                                    xt8t[:, j, ts(i, 128)],
                                    w8t[:, n, j, :],
                                    start=(j == 0), stop=(j == KCH - 1))
                        col = attn[i][:, n:n + 1]
                        # drain: acc += delta' * P, spread across engines
                        if n == 0:
                            nc.scalar.activation(acc[i][:], pp[:], AF.Copy,
                                                 scale=col)
                        elif (n * TPB + t) % 5 < 3:
                            nc.vector.scalar_tensor_tensor(
                                acc[i][:], pp[:], col, acc[i][:],
                                op0=ALU.mult, op1=ALU.add)
                        else:
                            sc = scp.tile([128, D], BF16, tag="sc", name="sc")
                            nc.scalar.activation(sc[:], pp[:], AF.Copy,
                                                 scale=col)
                            nc.gpsimd.tensor_add(acc[i][:], acc[i][:], sc[:])
                # block done: store bf16 partials + RS
                for t in range(TPB):
                    nc.sync.dma_start(partial[b][ts(t, 128), :],
                                      acc[i0 + t][:])
                nc.gpsimd.collective_compute(
                    "ReduceScatter",
                    ALU.add,
                    replica_groups=[list(range(NCORES))],
                    ins=[partial[b][:]],
                    outs=[rs_out[b][:]],
                )

            # ---- stage C: final projection per block ----
            identb = constp.tile([128, 128], BF16, tag="identb", name="identb")
            make_identity(nc, identb[:])
            for b in range(NBLK):
                yacc = constp.tile([128, D], BF16, tag=f"yacc{b}",
                                   name=f"yacc{b}")
                nc.sync.dma_start(yacc[:ROWS, :], rs_out[b][:])
                yt = [constp.tile([128, ROWS], BF16, tag=f"yt{b}_{e}",
                                  name=f"yt{b}_{e}") for e in range(KCH)]
                for e in range(KCH):
                    pt = pmain.tile([128, 128], BF16, tag="pm", name="pm")
                    nc.tensor.transpose(pt[:, :ROWS], yacc[:ROWS, ts(e, 128)],
                                        identb[:ROWS, :ROWS])
                    nc.vector.tensor_copy(yt[e][:, :ROWS], pt[:, :ROWS])
                po = pmain.tile([128, D], F32, tag="pm", name="pm")
                for e in range(KCH):
                    nc.tensor.matmul(po[:ROWS, :], yt[e][:, :ROWS], owtt[e][:],
                                     start=(e == 0), stop=False)
                for k in range(KCH):
                    nc.tensor.matmul(po[:ROWS, :], xst[b][k][:], m2t[k][:],
                                     start=False, stop=(k == KCH - 1))
                yo = constp.tile([128, D], F32, tag=f"yo{b}", name=f"yo{b}")
                nc.vector.tensor_add(yo[:ROWS, :], po[:ROWS, :],
                                     obbt[:ROWS, :])
                nc.sync.dma_start(y[ts(b, ROWS), :], yo[:ROWS, :])

    nc.compile()
    return nc


def kernel(x, positions, scales, value_weight, out_W, out_b):
    global _PROGRAM
    if _PROGRAM is None:
        _PROGRAM = _build_program()
    nc = _PROGRAM

    X = np.ascontiguousarray(np.asarray(x, np.float32).reshape(BT, D))
    XT = np.ascontiguousarray(X.T)                       # (D, BT)
    # [p, j, bt] = X[bt, j*128+p]
    xt8_h = np.ascontiguousarray(
        XT.reshape(KCH, 128, BT).transpose(1, 0, 2)
    ).astype(NPF8).reshape(128, KCH * BT)
    x2 = (X.astype(np.float64) ** 2).sum(1).astype(np.float32).reshape(BT, 1)
    pos = np.asarray(positions, np.float32)
    pn2 = (pos.astype(np.float64) ** 2).sum(1)           # (N,)
    sc = np.asarray(scales, np.float32)
    vw_full = np.asarray(value_weight, np.float32)
    owt = np.asarray(out_W, np.float32).T                # [e, d2]
    owtb_h = np.ascontiguousarray(owt).astype(NPBF)
    obb_h = np.tile(np.asarray(out_b, np.float32), (128, 1))
    wbar = vw_full.mean(axis=0, dtype=np.float64)        # (D, D)
    m2_h = np.ascontiguousarray(
        (wbar @ owt.astype(np.float64)).astype(np.float32))

    in_maps = []
    for c in range(NCORES):
        mine = np.arange(c * NLOC, (c + 1) * NLOC)
        rest = np.delete(np.arange(N), mine)
        perm = np.concatenate([mine, rest])
        posT = pos[perm].T                               # (D, N)
        pos8_h = np.ascontiguousarray(
            posT.reshape(KCH, 128, N).transpose(1, 0, 2)
        ).astype(NPF8).reshape(128, KCH * N)
        # [p, n, j, e] = 64 * W_perm[n][j*128+p, e]
        wloc = vw_full[mine] * WS                        # (16, 512, 512)
        w8_h = np.ascontiguousarray(
            wloc.reshape(NLOC, KCH, 128, D).transpose(2, 0, 1, 3)
        ).astype(NPF8).reshape(128, NLOC * KCH * D)
        # xs[b, k, p, r] = X[b*512 + c*64 + r, k*128 + p]
        xs_h = np.empty((NBLK, KCH, 128, ROWS), np.float32)
        for b in range(NBLK):
            sh = X[b * TPB * 128 + c * ROWS:
                   b * TPB * 128 + (c + 1) * ROWS, :]    # (64, 512)
            xs_h[b] = sh.T.reshape(KCH, 128, ROWS)
        in_maps.append({
            "xt8": xt8_h,
            "pos8": pos8_h,
            "aug": (-0.5 * pn2[perm]).astype(np.float32).astype(
                NPBF).reshape(1, N),
            "scb": np.tile(sc[perm], (128, 1)).astype(np.float32),
            "x2": x2,
            "w8": w8_h,
            "m2": m2_h,
            "owtb": owtb_h,
            "obb": obb_h,
            "xs": xs_h.reshape(NBLK * KCH * 128, ROWS),
        })

    trace = os.environ.get("BASS_KERNEL_TRACE", "0") == "1"
    res = run_bass_kernel_spmd(nc, in_maps, core_ids=list(range(NCORES)),
                               trace=trace)
    if trace:
        kernel.last_exec_time_ns = res.exec_time_ns
        kernel.last_trace = (res.instructions_and_trace or (None, None))[1]

    yfull = np.empty((BT, D), np.float32)
    for r in range(NCORES):
        yr = res.results[r]["y"]
        for b in range(NBLK):
            g0 = b * TPB * 128 + r * ROWS
            yfull[g0:g0 + ROWS] = yr[b * ROWS:(b + 1) * ROWS]
    return yfull.reshape(B, T, D)
